# revision 41
# baseline (speedup 1.0000x reference)
"""Trainium2 Bass kernel for nn_BlockCrossAttention (B=4,H=64,S=64,L=4,D=1024,NH=16).

Sharding: core c in 0..7 -> (b = c//2, half = c%2); 32 query/head rows per core.
Host prep (not counted in HW time): dtype casts + layout packs only --
  kx fp8-e4m3 in DoubleRow pooling layout (4 x 2MB chunks, 16KB lines),
  vx bf16 (4 x 1MB chunks), weights bf16 pre-transposed/chunked, qT
  pre-arranged, block-diag "ones" reducers.
On-chip: k mean-pool via PE DoubleRow matmuls accumulating in PSUM (fp32),
v sum-pool via DVE adds + PE fold, pair AllGather of pooled tensors (bf16),
bf16 projections/attention, softmax+entropy stats on full 128 partitions
(scores laid out [(l,q), (n,h)]), entropy-gated fusion via PE fold,
faithful-reshape scramble, o_proj. Output rows disjoint across cores.
DMA spread over sync/gpsimd/scalar queues: big inputs first, weights behind.
"""

import numpy as np

B, H, S, L, D = 4, 64, 64, 4, 1024
NH, DH = 16, 64
NCORES = 8

_CACHE = {}
LAST_RESULTS = None  # test.py reads exec_time from here


def _build_nc(stage=99):
    import concourse.bacc as bacc
    import concourse.bass as bass
    import concourse.tile as tile
    from concourse import mybir
    from concourse.masks import make_identity

    f32 = mybir.dt.float32
    bf16 = mybir.dt.bfloat16
    f8e4 = mybir.dt.float8e4
    AF = mybir.ActivationFunctionType
    AX = mybir.AxisListType
    OP = mybir.AluOpType
    DR = mybir.MatmulPerfMode.DoubleRow

    nc = bacc.Bacc("TRN2", target_bir_lowering=False, debug=False, num_devices=NCORES)

    kx = nc.dram_tensor("kx", [512, 16384], f8e4, kind="ExternalInput")
    vx = nc.dram_tensor("vx", [512, 4096], bf16, kind="ExternalInput")
    qT = nc.dram_tensor("qT", [128, 256], bf16, kind="ExternalInput")
    W = {w: nc.dram_tensor(w, [128, 8192], bf16, kind="ExternalInput")
         for w in ("WqT", "WkT", "WvT", "WoT")}
    bq = nc.dram_tensor("bq", [1024], f32, kind="ExternalInput")
    bkT2 = nc.dram_tensor("bkT2", [128, 8], f32, kind="ExternalInput")
    bv = nc.dram_tensor("bv", [1024], f32, kind="ExternalInput")
    bo = nc.dram_tensor("bo", [1024], f32, kind="ExternalInput")
    onesk = nc.dram_tensor("onesk", [128, 64], f8e4, kind="ExternalInput")
    onesv = nc.dram_tensor("onesv", [128, 32], bf16, kind="ExternalInput")
    onesf = nc.dram_tensor("onesf", [128, 32], bf16, kind="ExternalInput")
    onesb = nc.dram_tensor("onesb", [32, 64], bf16, kind="ExternalInput")
    ident = nc.dram_tensor("ident", [128, 128], bf16, kind="ExternalInput")
    out_d = nc.dram_tensor("out", [32, 1024], f32, kind="ExternalOutput")
    cc_out_sh = nc.dram_tensor("cc_out_sh", [64, 5120], bf16, kind="Internal",
                               addr_space="Shared")

    with tile.TileContext(nc) as tc:
        _emit(nc, tc, bass, mybir, f32, bf16, f8e4, AF, AX, OP, DR,
              make_identity, kx, vx, qT, W, bq, bkT2, bv, bo,
              onesk, onesv, onesf, onesb, ident, out_d, cc_out_sh, stage)
    nc.compile()
    return nc


def _emit(nc, tc, bass, mybir, f32, bf16, f8e4, AF, AX, OP, DR, make_identity,
          kx, vx, qT, W, bq, bkT2_d, bv, bo, onesk_d, onesv_d, onesf_d,
          onesb_d, ident_d, out_d, cc_out_sh, stage=99):
    stack = []

    def popen(cm):
        stack.append(cm)
        return cm.__enter__()

    def pclose(cm):
        assert stack and stack[-1] is cm, "pool close order"
        stack.pop()
        cm.__exit__(None, None, None)

    def finish():
        for cm in reversed(stack[:]):
            pclose(cm)

    def bcast(dst_ap, src_t, n):
        ap = src_t.ap()
        nc.gpsimd.dma_start(out=dst_ap, in_=bass.AP(
            tensor=ap.tensor, offset=ap.offset, ap=[[0, n]] + list(ap.ap)))

    def dbg_out(src_ap):
        dbg_cm = popen(tc.tile_pool(name="dbg", bufs=1))
        dbg = dbg_cm.tile([32, 1024], f32, tag="dbg")
        nc.vector.tensor_copy(out=dbg[:], in_=src_ap)
        nc.sync.dma_start(out=out_d[:, :], in_=dbg[:])
        finish()

    consts = popen(tc.tile_pool(name="consts", bufs=1))
    keep = popen(tc.tile_pool(name="keep", bufs=1))
    wt = popen(tc.tile_pool(name="wt", bufs=1))
    dram = popen(tc.tile_pool(name="dram", bufs=1, space="DRAM"))
    p1 = popen(tc.tile_pool(name="p1", bufs=1))

    # ---------------- DMA schedule ----------------------------------------
    # gpsimd queue: tiny consts first (pooling needs them), then kx g2,g3,
    #   then bias broadcasts.  sync queue: kx g0,g1, later cc staging.
    # scalar queue: vx 0..3, qT, then weights Wq,Wk,Wv,Wo (needed later).
    onesk_t = consts.tile([128, 64], f8e4, tag="onesk")
    onesv_t = consts.tile([128, 32], bf16, tag="onesv")
    onesf_t = consts.tile([128, 32], bf16, tag="onesf")
    onesb_t = consts.tile([32, 64], bf16, tag="onesb")
    ident_t = consts.tile([128, 128], bf16, tag="ident")
    bkT2 = consts.tile([128, 8], f32, tag="bkT2")
    nc.gpsimd.dma_start(out=onesk_t[:], in_=onesk_d[:, :])
    nc.gpsimd.dma_start(out=onesv_t[:], in_=onesv_d[:, :])
    nc.gpsimd.dma_start(out=onesf_t[:], in_=onesf_d[:, :])
    nc.gpsimd.dma_start(out=onesb_t[:], in_=onesb_d[:, :])
    nc.gpsimd.dma_start(out=ident_t[:], in_=ident_d[:, :])
    nc.gpsimd.dma_start(out=bkT2[:], in_=bkT2_d[:, :])

    kxr = kx.ap().rearrange("(g p) f -> g p f", p=128)
    vxr = vx.ap().rearrange("(g p) f -> g p f", p=128)
    kxp_cm = tc.tile_pool(name="kxp", bufs=4)
    kxp = popen(kxp_cm)
    vxp_cm = tc.tile_pool(name="vxp", bufs=2)
    vxp = popen(vxp_cm)
    qT_sb = consts.tile([128, 256], bf16, tag="qT_sb")
    nc.scalar.dma_start(out=qT_sb[:], in_=qT[:, :])
    wsb = {}
    for wname in ("WqT", "WkT", "WvT", "WoT"):
        wsb[wname] = wt.tile([128, 8192], bf16, tag=wname, name=wname)
    nc.scalar.dma_start(out=wsb["WqT"][:], in_=W["WqT"][:, :])
    kts = []
    for g in range(4):
        kt = kxp.tile([128, 16384], f8e4, tag="kt", name=f"kt{g}")
        eng = nc.sync if g % 2 == 0 else nc.gpsimd
        eng.dma_start(out=kt[:], in_=kxr[g])
        kts.append(kt)
    vts = []
    for g in range(4):
        vt = vxp.tile([128, 4096], bf16, tag="vt", name=f"vt{g}")
        nc.scalar.dma_start(out=vt[:], in_=vxr[g])
        vts.append(vt)
    for wname in ("WkT", "WvT", "WoT"):
        nc.scalar.dma_start(out=wsb[wname][:], in_=W[wname][:, :])
    bq_bc = consts.tile([32, 1024], f32, tag="bq_bc")
    bv_bc = consts.tile([64, 1024], f32, tag="bv_bc")
    bo_bc = consts.tile([32, 1024], f32, tag="bo_bc")
    bcast(bq_bc[:], bq, 32)
    bcast(bv_bc[:], bv, 64)
    bcast(bo_bc[:], bo, 32)

    # ---------------- phase D: q projection + transpose (overlaps CC) ----
    pq_cm = tc.tile_pool(name="pq", bufs=1, space="PSUM")
    pq = popen(pq_cm)
    ps_q = pq.tile([32, 1024], f32, tag="ps_q")
    for c in range(8):
        for ch in range(2):
            nc.tensor.matmul(ps_q[:, 512 * ch:512 * (ch + 1)],
                             qT_sb[:, 32 * c:32 * (c + 1)],
                             wsb["WqT"][:, 1024 * c + 512 * ch:
                                        1024 * c + 512 * (ch + 1)],
                             start=(c == 0), stop=(c == 7))
    _q_sb = p1.tile([32, 1024], bf16, tag="_q_sb")
    nc.vector.tensor_add(_q_sb[:], ps_q[:], bq_bc[:])
    # qTt3 [128, 512]: col n*32+q, head n at rows 64*(n%2); other half zero
    qTt3 = keep.tile([128, 512], bf16, tag="qTt3")
    nc.vector.memset(qTt3[:], 0.0)
    qt_ps = pq.tile([128, 512], bf16, tag="qt_ps")
    for n in range(16):
        r0 = 64 * (n % 2)
        nc.tensor.transpose(qt_ps[r0:r0 + 64, 32 * n:32 * (n + 1)],
                            _q_sb[:, 64 * n:64 * (n + 1)], ident_t[:32, :32])
    for n in range(16):
        r0 = 64 * (n % 2)
        nc.vector.tensor_copy(out=qTt3[r0:r0 + 64, 32 * n:32 * (n + 1)],
                              in_=qt_ps[r0:r0 + 64, 32 * n:32 * (n + 1)])
    pclose(pq_cm)

    # ---------------- phase B: k mean-pool (PE DoubleRow, per-level) -----
    kpool_bf = keep.tile([32, 4096], bf16, tag="kpool_bf")   # (h, l*1024+d)
    ppk_cm = tc.tile_pool(name="ppk", bufs=2, space="PSUM")
    ppk = popen(ppk_cm)
    lhs_k = onesk_t[:].rearrange("p (i m) -> p i m", i=2)
    kt4 = [kts[g][:].rearrange("p (hf l i f) -> p hf l i f", hf=2, l=4, i=2)
           for g in range(4)]
    for l in range(4):
        kp_ps = ppk.tile([32, 1024], f32, tag="kp_ps", name=f"kp_ps{l}")
        for g in range(4):
            for hf in range(2):
                for bk_ in range(2):
                    nc.tensor.matmul(
                        kp_ps[:, 512 * bk_:512 * (bk_ + 1)], lhs_k,
                        kt4[g][:, hf, l, :, 512 * bk_:512 * (bk_ + 1)],
                        start=(g == 0 and hf == 0),
                        stop=(g == 3 and hf == 1), perf_mode=DR)
        nc.vector.tensor_copy(out=kpool_bf[:, 1024 * l:1024 * (l + 1)],
                              in_=kp_ps[:])
    pclose(ppk_cm)

    # ---------------- phase C: v sum-pool (DVE adds + PE fold) -----------
    vpool_bf = keep.tile([32, 1024], bf16, tag="vpool_bf")   # sum_s v[l=3]
    vacc = p1.tile([128, 1024], f32, tag="vacc")
    for g in range(4):
        for j in range(4):
            if j >= 4 or 1024 * (j + 1) > 4096:
                continue
            sl = vts[g][:, 1024 * j:1024 * (j + 1)]
            if g == 0 and j == 0:
                nc.vector.tensor_copy(out=vacc[:], in_=sl)
            else:
                nc.vector.tensor_add(vacc[:], vacc[:], sl)
    vacc_bf = p1.tile([128, 1024], bf16, tag="vacc_bf")
    nc.vector.tensor_copy(out=vacc_bf[:], in_=vacc[:])
    ppv_cm = tc.tile_pool(name="ppv", bufs=1, space="PSUM")
    ppv = popen(ppv_cm)
    vp_ps = ppv.tile([32, 1024], f32, tag="vp_ps")
    for ch in range(2):
        nc.tensor.matmul(vp_ps[:, 512 * ch:512 * (ch + 1)], onesv_t[:],
                         vacc_bf[:, 512 * ch:512 * (ch + 1)],
                         start=True, stop=True)
    nc.vector.tensor_copy(out=vpool_bf[:], in_=vp_ps[:])
    pclose(ppv_cm)
    pclose(vxp_cm)
    pclose(kxp_cm)

    if stage < 2:
        dbg_out(vpool_bf[:])
        return

    # ---------------- phase E: pair AllGather of pooled tensors ----------
    cc_in = dram.tile([32, 5120], bf16, tag="cc_in")
    cc_out = dram.tile([64, 5120], bf16, tag="cc_out")
    nc.sync.dma_start(out=cc_in[:, :4096], in_=kpool_bf[:])
    nc.sync.dma_start(out=cc_in[:, 4096:], in_=vpool_bf[:])
    nc.gpsimd.collective_compute(
        "AllGather", mybir.AluOpType.bypass,
        replica_groups=[[0, 1], [2, 3], [4, 5], [6, 7]],
        ins=[cc_in[:].opt()], outs=[cc_out[:].opt()])

    # ---------------- phase F: gather-back + kpT/vpT transposes ----------
    kpall = p1.tile([64, 4096], bf16, tag="kpall")
    vpall = p1.tile([64, 1024], bf16, tag="vpall")
    nc.sync.dma_start(out=kpall[:], in_=cc_out[:, :4096])
    nc.sync.dma_start(out=vpall[:], in_=cc_out[:, 4096:])

    kpT = keep.tile([128, 2048], bf16, tag="kpT")   # [dd, c*256 + l*64 + h]
    vpT = keep.tile([128, 512], bf16, tag="vpT")    # [dd, c*64 + h]
    pt_cm = tc.tile_pool(name="pt", bufs=2, space="PSUM")
    pt = popen(pt_cm)
    for c in range(8):
        tp = pt.tile([128, 256], bf16, tag="ptr")
        for l in range(4):
            nc.tensor.transpose(tp[:, 64 * l:64 * (l + 1)],
                                kpall[:, 1024 * l + 128 * c:
                                      1024 * l + 128 * (c + 1)],
                                ident_t[:64, :64])
        nc.vector.tensor_copy(out=kpT[:, 256 * c:256 * (c + 1)], in_=tp[:])
    for c in range(8):
        tpv = pt.tile([128, 64], bf16, tag="ptrv")
        nc.tensor.transpose(tpv[:], vpall[:, 128 * c:128 * (c + 1)],
                            ident_t[:64, :64])
        nc.vector.tensor_copy(out=vpT[:, 64 * c:64 * (c + 1)], in_=tpv[:])

    if stage < 3:
        dbg_out(kpT[:32, :1024])
        return

    # ---------------- phase G: kbT3 = (kp @ WkT).T + bk  (bf16) ----------
    # kbT3 [128, 4096]: col n*256 + l*64 + h, head n at rows 64*(n%2),
    # other 64 rows zero (scores contract full 128 partitions from base 0).
    kbT3 = keep.tile([128, 4096], bf16, tag="kbT3")
    nc.vector.memset(kbT3[:], 0.0)
    pwg_cm = tc.tile_pool(name="pwg", bufs=2, space="PSUM")
    pwg = popen(pwg_cm)
    for jj in range(8):
        ps = pwg.tile([128, 256], f32, tag="pws")
        for c in range(8):
            nc.tensor.matmul(ps[:],
                             wsb["WkT"][:, 1024 * c + 128 * jj:
                                        1024 * c + 128 * (jj + 1)],
                             kpT[:, 256 * c:256 * (c + 1)],
                             start=(c == 0), stop=(c == 7))
        for ip in range(2):
            n = 2 * jj + ip
            r0 = 64 * ip
            nc.vector.tensor_scalar_add(
                out=kbT3[r0:r0 + 64, 256 * n:256 * (n + 1)],
                in0=ps[r0:r0 + 64, :], scalar1=bkT2[r0:r0 + 64, jj:jj + 1])
    pclose(pwg_cm)

    if stage < 31:
        dbg_out(kbT3[:32, :1024])
        return

    # ---------------- phase H: v_blk = vp @ WvT + 64*bv  [h, o] bf16 -----
    pwh_cm = tc.tile_pool(name="pwh", bufs=1, space="PSUM")
    pwh = popen(pwh_cm)
    vb_sb = keep.tile([64, 1024], bf16, tag="vb_sb")
    ps_vb = pwh.tile([64, 1024], f32, tag="ps_vb")
    for c in range(8):
        for ch in range(2):
            nc.tensor.matmul(ps_vb[:, 512 * ch:512 * (ch + 1)],
                             vpT[:, 64 * c:64 * (c + 1)],
                             wsb["WvT"][:, 1024 * c + 512 * ch:
                                        1024 * c + 512 * (ch + 1)],
                             start=(c == 0), stop=(c == 7))
    nc.vector.scalar_tensor_tensor(out=vb_sb[:], in0=bv_bc[:], scalar=64.0,
                                   in1=ps_vb[:], op0=OP.mult, op1=OP.add)
    pclose(pwh_cm)

    if stage < 32:
        dbg_out(vb_sb[:32, :])
        return

    # ---------------- phase I: scores fp32 ------------------------------
    # layout [64, 2048]: partition p = 32*(l%2) + q, col (l//2)*1024 + n*64 + h
    p2 = popen(tc.tile_pool(name="p2", bufs=1))
    st = popen(tc.tile_pool(name="stats", bufs=1))
    pwi_cm = tc.tile_pool(name="pwi", bufs=1, space="PSUM")
    pwi = popen(pwi_cm)
    scps = pwi.tile([64, 2048], f32, tag="scps")
    for l in range(4):
        p0, c0 = 32 * (l % 2), 1024 * (l // 2)
        for n in range(16):
            nc.tensor.matmul(scps[p0:p0 + 32, c0 + 64 * n:c0 + 64 * (n + 1)],
                             qTt3[:, 32 * n:32 * (n + 1)],
                             kbT3[:, 256 * n + 64 * l:256 * n + 64 * (l + 1)],
                             start=True, stop=True)
    if stage < 4:
        dbg_out(scores[:32, :1024])
        return

    # ---------------- phase J: softmax + entropy + fusion ----------------
    # p = 32*lp + q, col cb*1024 + n*64 + h, l = 2*cb + lp
    # scores stay in PSUM (scps, pre-scale); P = exp(0.125*scps) via ACT,
    # sP = (0.125*scps)*P via GpSimd -- no SBUF scores copy at all.
    P_sb = p2.tile([64, 2048], f32, tag="P_sb")
    nc.scalar.activation(out=P_sb[:], in_=scps[:], func=AF.Exp, scale=0.125)
    sP = p2.tile([64, 2048], f32, tag="sP")
    nc.vector.scalar_tensor_tensor(out=sP[:], in0=scps[:], scalar=0.125,
                                   in1=P_sb[:], op0=OP.mult, op1=OP.mult)
    Z = st.tile([64, 32], f32, tag="Z")
    nc.vector.reduce_sum(Z[:], P_sb[:].rearrange("p (m h) -> p m h", h=64), AX.X)
    S2 = st.tile([64, 32], f32, tag="S2")
    nc.vector.reduce_sum(S2[:], sP[:].rearrange("p (m h) -> p m h", h=64), AX.X)
    rZ = st.tile([64, 32], f32, tag="rZ")
    nc.vector.reciprocal(rZ[:], Z[:])
    logZ = st.tile([64, 32], f32, tag="logZ")
    nc.scalar.activation(out=logZ[:], in_=Z[:], func=AF.Ln)
    pclose(pwi_cm)

    if stage < 4:
        dbg_out(P_sb[:32, :1024])
        return

    Hent = st.tile([64, 32], f32, tag="Hent")   # entropy per p x (cb, n)
    nc.vector.tensor_mul(Hent[:], S2[:], rZ[:])
    nc.vector.tensor_sub(Hent[:], logZ[:], Hent[:])
    Hsum = st.tile([64, 2], f32, tag="Hsum")    # sum_n -> per (lp,q) x cb
    nc.vector.reduce_sum(Hsum[:], Hent[:].rearrange("p (m n) -> p m n", n=16),
                         AX.X)
    # w_lvl = softmax over l = (2*cb + lp): partial sums via PE pair-folds
    eH = st.tile([64, 2], f32, tag="eH")
    nc.scalar.activation(out=eH[:], in_=Hsum[:], func=AF.Exp,
                         scale=-1.0 / (16.0 * float(np.log(64.0))))
    E1 = st.tile([64, 1], bf16, tag="E1")       # sum over cb
    nc.vector.tensor_add(E1[:], eH[:, 0:1], eH[:, 1:2])
    pj_cm = tc.tile_pool(name="pj", bufs=1, space="PSUM")
    pj = popen(pj_cm)
    E2_ps = pj.tile([32, 1], f32, tag="E2_ps")  # sum over lp (partition pairs)
    nc.tensor.matmul(E2_ps[:], onesf_t[:64, :], E1[:], start=True, stop=True)
    E2_sb = st.tile([32, 1], bf16, tag="E2_sb")
    nc.vector.tensor_copy(out=E2_sb[:], in_=E2_ps[:])
    Eb_ps = pj.tile([64, 1], f32, tag="Eb_ps")  # broadcast back to both lp
    nc.tensor.matmul(Eb_ps[:], onesb_t[:], E2_sb[:], start=True, stop=True)
    rE = st.tile([64, 1], f32, tag="rE")
    nc.vector.reciprocal(rE[:], Eb_ps[:])
    wl = st.tile([64, 2], f32, tag="wl")        # softmax weight for l=2cb+lp
    nc.vector.tensor_scalar_mul(out=wl[:], in0=eH[:], scalar1=rE[:, :1])
    # g[p, (cb,n)] = wl[p, cb] * rZ[p, (cb,n)]; P~ = P * g (bf16)
    g = st.tile([64, 32], f32, tag="g")
    wl_ap = wl[:]
    wl_b = bass.AP(tensor=wl_ap.tensor, offset=wl_ap.offset,
                   ap=[wl_ap.ap[0], wl_ap.ap[1], [0, 16]])
    nc.vector.tensor_mul(g[:].rearrange("p (m n) -> p m n", n=16),
                         rZ[:].rearrange("p (m n) -> p m n", n=16), wl_b)
    Pt_bf = p2.tile([64, 2048], bf16, tag="Pt_bf")
    g_ap = g[:]
    for eng, half in ((nc.vector, 0), (nc.gpsimd, 1)):
        g_h = bass.AP(tensor=g_ap.tensor, offset=g_ap.offset + 16 * half,
                      ap=[g_ap.ap[0], [g_ap.ap[1][0], 16], [0, 64]])
        eng.tensor_mul(
            Pt_bf[:, 1024 * half:1024 * (half + 1)].rearrange(
                "p (m h) -> p m h", h=64),
            P_sb[:, 1024 * half:1024 * (half + 1)].rearrange(
                "p (m h) -> p m h", h=64), g_h)
    # fused[q, (n,h)] = sum over lp (pair-fold) and cb (psum accumulate)
    fused_ps = pj.tile([32, 1024], f32, tag="fused_ps")
    for cb in range(2):
        for ch in range(2):
            nc.tensor.matmul(fused_ps[:, 512 * ch:512 * (ch + 1)],
                             onesf_t[:64, :],
                             Pt_bf[:, 1024 * cb + 512 * ch:
                                   1024 * cb + 512 * (ch + 1)],
                             start=(cb == 0), stop=(cb == 1))
    fused_bf = st.tile([32, 1024], bf16, tag="fused_bf")
    nc.vector.tensor_copy(out=fused_bf[:], in_=fused_ps[:])
    pclose(pj_cm)

    # ---------------- phase K: ctx = fused @ v_blk ----------------------
    pwk_cm = tc.tile_pool(name="pwk", bufs=1, space="PSUM")
    pwk = popen(pwk_cm)
    fusedT = st.tile([64, 512], bf16, tag="fusedT")  # col n*32 + q
    ps_ft = pwk.tile([64, 512], bf16, tag="ps_ft")
    for n in range(16):
        nc.tensor.transpose(ps_ft[:, 32 * n:32 * (n + 1)],
                            fused_bf[:, 64 * n:64 * (n + 1)], ident_t[:32, :32])
    nc.vector.tensor_copy(out=fusedT[:], in_=ps_ft[:])
    ps_ctx = pwk.tile([32, 1024], f32, tag="ps_ctx")
    for n in range(16):
        nc.tensor.matmul(ps_ctx[:, 64 * n:64 * (n + 1)],
                         fusedT[:, 32 * n:32 * (n + 1)],
                         vb_sb[:, 64 * n:64 * (n + 1)], start=True, stop=True)
    ctx_sb = st.tile([32, 1024], bf16, tag="ctx_sb")
    for cg in range(4):
        nc.vector.tensor_copy(out=ctx_sb[:, 256 * cg:256 * (cg + 1)],
                              in_=ps_ctx[:, 256 * cg:256 * (cg + 1)])
    pclose(pwk_cm)

    # ---------------- phase L: faithful-reshape scramble + o_proj --------
    # Y[r=(2n+jp), qq*64+dh] = ctx[16*jp+qq, 64*n+dh]; spread over 3 queues
    Y_sb = st.tile([32, 1024], bf16, tag="Y_sb")
    for n in range(16):
        eng = (nc.gpsimd, nc.sync, nc.scalar)[n % 3]
        eng.dma_start(out=Y_sb[2 * n:2 * n + 2, :],
                      in_=ctx_sb[:, 64 * n:64 * (n + 1)])
    pwl_cm = tc.tile_pool(name="pwl", bufs=1, space="PSUM")
    pwl = popen(pwl_cm)
    YT = st.tile([128, 256], bf16, tag="YT")  # col mc*32+r, row mm
    for mg in range(2):
        tp = pt.tile([128, 256], bf16, tag="ptr")
        for j in range(4):
            mc = 4 * mg + j
            nc.tensor.transpose(tp[:, 32 * j:32 * (j + 1)],
                                Y_sb[:, 128 * mc:128 * (mc + 1)], ident_t[:32, :32])
        nc.vector.tensor_copy(out=YT[:, 128 * mg:128 * (mg + 1)], in_=tp[:, :128])

    out_sb = st.tile([32, 1024], f32, tag="out_sb")
    ps_o = pwl.tile([32, 1024], f32, tag="ps_o")
    for ch in range(2):
        for mc in range(8):
            nc.tensor.matmul(ps_o[:, 512 * ch:512 * (ch + 1)],
                             YT[:, 32 * mc:32 * (mc + 1)],
                             wsb["WoT"][:, 1024 * mc + 512 * ch:
                                        1024 * mc + 512 * (ch + 1)],
                             start=(mc == 0), stop=(mc == 7))
    nc.vector.tensor_add(out_sb[:], ps_o[:], bo_bc[:])
    nc.sync.dma_start(out=out_d[:, :], in_=out_sb[:])

    finish()


def _get_nc():
    if "nc" not in _CACHE:
        _CACHE["nc"] = _build_nc()
    return _CACHE["nc"]


def make_in_maps(q, k, v, Wq, bq, Wk, bk, Wv, bv, Wo, bo):
    import ml_dtypes
    f8 = ml_dtypes.float8_e4m3
    bf = ml_dtypes.bfloat16

    q = np.asarray(q, np.float32)
    k = np.asarray(k, np.float32)
    v = np.asarray(v, np.float32)
    Ws = {"Wq": np.asarray(Wq, np.float32), "Wk": np.asarray(Wk, np.float32),
          "Wv": np.asarray(Wv, np.float32), "Wo": np.asarray(Wo, np.float32)}
    bs = {"bq": np.asarray(bq, np.float32), "bk": np.asarray(bk, np.float32),
          "bv": np.asarray(bv, np.float32), "bo": np.asarray(bo, np.float32)}

    # shared across cores
    WT = {}
    for n_, w in Ws.items():
        WT[n_ + "T"] = np.ascontiguousarray(
            w.T.reshape(8, 128, 1024).transpose(1, 0, 2).reshape(128, 8192)
        ).astype(bf)
    bkT2 = np.ascontiguousarray(bs["bk"].reshape(8, 128).T)  # [128, 8]
    pidx = np.arange(128)
    onesk = np.zeros((128, 64), np.float32)
    for i in range(2):
        onesk[pidx, i * 32 + pidx // 4] = 1.0 / 64.0
    onesk = onesk.astype(f8)
    onesv = np.zeros((128, 32), np.float32)
    onesv[pidx, pidx // 4] = 1.0
    onesv = onesv.astype(bf)
    onesf = np.zeros((128, 32), np.float32)
    onesf[pidx, pidx % 32] = 1.0
    onesf = onesf.astype(bf)
    onesb = np.ascontiguousarray(onesf[:64].T).astype(bf)
    ident = np.eye(128, dtype=np.float32).astype(bf)

    in_maps = []
    for c in range(NCORES):
        b, half = c // 2, c % 2
        hs = slice(32 * half, 32 * half + 32)
        # kx rows g*128 + (4h+sa), cols hf*8192 + l*2048 + i*1024 + d,
        # with s = (2g+hf)*8 + sa*2 + i
        kc = k[b, hs]                                  # (32, 64, 4, 1024)
        kc = kc.reshape(32, 4, 2, 4, 2, 4, 1024)       # h, g, hf, sa, i, l, d
        kc = kc.transpose(1, 0, 3, 2, 5, 4, 6).reshape(512, 16384)
        # vx rows g*128 + (4h+sa), cols j*1024 + d, s = (g*4+j)*4 + sa
        vc = v[b, hs, :, L - 1, :]                     # (32, 64, 1024)
        vc = vc.reshape(32, 4, 4, 4, 1024)             # h, g, j, sa, d
        vc = vc.transpose(1, 0, 3, 2, 4).reshape(512, 4096)
        # qT: [dd, c*32 + r]
        qc = q[b, hs]                                  # (32, 1024)
        qTc = qc.T.reshape(8, 128, 32).transpose(1, 0, 2).reshape(128, 256)
        in_maps.append(dict(
            kx=np.ascontiguousarray(kc).astype(f8),
            vx=np.ascontiguousarray(vc).astype(bf),
            qT=np.ascontiguousarray(qTc).astype(bf),
            **WT, bq=bs["bq"], bkT2=bkT2, bv=bs["bv"], bo=bs["bo"],
            onesk=onesk, onesv=onesv, onesf=onesf, onesb=onesb,
            ident=ident))
    return in_maps


def assemble(results):
    out = np.empty((B, H, D), np.float32)
    for c in range(NCORES):
        b, half = c // 2, c % 2
        o = results[c]["out"]  # rows r = 2n + jp  ->  h' = 4n + 2*half + jp
        for r in range(32):
            out[b, 4 * (r // 2) + 2 * half + (r % 2)] = o[r]
    return out


def _install_ntff_shim():
    """Register the axon NTFF profile hook if the image's antenv lacks it."""
    import sys
    import types
    try:
        if "antenv.axon_hooks" in sys.modules:
            return
        import antenv
        mod = types.ModuleType("antenv.axon_hooks")
        mod._hook = None

        def set_axon_ntff_profile_hook(h):
            mod._hook = h

        def get_axon_ntff_profile_hook():
            return mod._hook

        mod.set_axon_ntff_profile_hook = set_axon_ntff_profile_hook
        mod.get_axon_ntff_profile_hook = get_axon_ntff_profile_hook
        sys.modules["antenv.axon_hooks"] = mod
        antenv.axon_hooks = mod
        from trn_agent_boot.trn_boot import _ntff_profile_via_ctypes
        hook = _ntff_profile_via_ctypes("/opt/axon/libaxon_pjrt.so")
        if hook is not None:
            set_axon_ntff_profile_hook(hook)
    except Exception:
        pass  # tracing degrades; execution unaffected


def kernel(q, k, v, Wq, bq, Wk, bk, Wv, bv, Wo, bo, _trace=False):
    global LAST_RESULTS
    from concourse.bass_utils import run_bass_kernel_spmd
    if _trace:
        _install_ntff_shim()
    nc = _get_nc()
    in_maps = make_in_maps(q, k, v, Wq, bq, Wk, bk, Wv, bv, Wo, bo)
    res = run_bass_kernel_spmd(nc, in_maps, list(range(NCORES)), trace=_trace)
    LAST_RESULTS = res
    return assemble(res.results)


# revision 42
# speedup vs baseline: 1.1049x; 1.1049x over previous
"""Trainium2 Bass kernel for nn_BlockCrossAttention (B=4,H=64,S=64,L=4,D=1024,NH=16).

Sharding: core c in 0..7 -> (b = c//2, half = c%2); 32 query/head rows per core.
Host prep (not counted in HW time): dtype casts + layout packs only --
  kx fp8-e4m3 in DoubleRow pooling layout (4 x 2MB chunks, 16KB lines),
  vx bf16 (4 x 1MB chunks), weights bf16 pre-transposed/chunked, qT
  pre-arranged, block-diag "ones" reducers.
On-chip: k mean-pool via PE DoubleRow matmuls accumulating in PSUM (fp32),
v sum-pool via DVE adds + PE fold, pair AllGather of pooled tensors (bf16),
bf16 projections/attention, softmax+entropy stats on full 128 partitions
(scores laid out [(l,q), (n,h)]), entropy-gated fusion via PE fold,
faithful-reshape scramble, o_proj. Output rows disjoint across cores.
DMA spread over sync/gpsimd/scalar queues: big inputs first, weights behind.
"""

import numpy as np

B, H, S, L, D = 4, 64, 64, 4, 1024
NH, DH = 16, 64
NCORES = 8

_CACHE = {}
LAST_RESULTS = None  # test.py reads exec_time from here


def _build_nc(stage=99):
    import concourse.bacc as bacc
    import concourse.bass as bass
    import concourse.tile as tile
    from concourse import mybir
    from concourse.masks import make_identity

    f32 = mybir.dt.float32
    bf16 = mybir.dt.bfloat16
    f8e4 = mybir.dt.float8e4
    AF = mybir.ActivationFunctionType
    AX = mybir.AxisListType
    OP = mybir.AluOpType
    DR = mybir.MatmulPerfMode.DoubleRow

    nc = bacc.Bacc("TRN2", target_bir_lowering=False, debug=False, num_devices=NCORES)

    kx = nc.dram_tensor("kx", [512, 16384], f8e4, kind="ExternalInput")
    vx = nc.dram_tensor("vx", [512, 4096], bf16, kind="ExternalInput")
    qT = nc.dram_tensor("qT", [128, 256], bf16, kind="ExternalInput")
    W = {w: nc.dram_tensor(w, [128, 8192], bf16, kind="ExternalInput")
         for w in ("WqT", "WkT", "WvT", "WoT")}
    bq = nc.dram_tensor("bq", [1024], f32, kind="ExternalInput")
    bkT2 = nc.dram_tensor("bkT2", [128, 8], f32, kind="ExternalInput")
    bv = nc.dram_tensor("bv", [1024], f32, kind="ExternalInput")
    bo = nc.dram_tensor("bo", [1024], f32, kind="ExternalInput")
    onesk = nc.dram_tensor("onesk", [128, 64], f8e4, kind="ExternalInput")
    onesv = nc.dram_tensor("onesv", [128, 32], bf16, kind="ExternalInput")
    onesf = nc.dram_tensor("onesf", [128, 32], bf16, kind="ExternalInput")
    onesb = nc.dram_tensor("onesb", [32, 64], bf16, kind="ExternalInput")
    ident = nc.dram_tensor("ident", [128, 128], bf16, kind="ExternalInput")
    out_d = nc.dram_tensor("out", [32, 1024], f32, kind="ExternalOutput")
    cc_out_sh = nc.dram_tensor("cc_out_sh", [64, 5120], bf16, kind="Internal",
                               addr_space="Shared")

    with tile.TileContext(nc) as tc:
        _emit(nc, tc, bass, mybir, f32, bf16, f8e4, AF, AX, OP, DR,
              make_identity, kx, vx, qT, W, bq, bkT2, bv, bo,
              onesk, onesv, onesf, onesb, ident, out_d, cc_out_sh, stage)
    nc.compile()
    return nc


def _emit(nc, tc, bass, mybir, f32, bf16, f8e4, AF, AX, OP, DR, make_identity,
          kx, vx, qT, W, bq, bkT2_d, bv, bo, onesk_d, onesv_d, onesf_d,
          onesb_d, ident_d, out_d, cc_out_sh, stage=99):
    stack = []

    def popen(cm):
        stack.append(cm)
        return cm.__enter__()

    def pclose(cm):
        assert stack and stack[-1] is cm, "pool close order"
        stack.pop()
        cm.__exit__(None, None, None)

    def finish():
        for cm in reversed(stack[:]):
            pclose(cm)

    def bcast(dst_ap, src_t, n):
        ap = src_t.ap()
        nc.gpsimd.dma_start(out=dst_ap, in_=bass.AP(
            tensor=ap.tensor, offset=ap.offset, ap=[[0, n]] + list(ap.ap)))

    def dbg_out(src_ap):
        dbg_cm = popen(tc.tile_pool(name="dbg", bufs=1))
        dbg = dbg_cm.tile([32, 1024], f32, tag="dbg")
        nc.vector.tensor_copy(out=dbg[:], in_=src_ap)
        nc.sync.dma_start(out=out_d[:, :], in_=dbg[:])
        finish()

    consts = popen(tc.tile_pool(name="consts", bufs=1))
    keep = popen(tc.tile_pool(name="keep", bufs=1))
    wt = popen(tc.tile_pool(name="wt", bufs=1))
    dram = popen(tc.tile_pool(name="dram", bufs=1, space="DRAM"))
    p1 = popen(tc.tile_pool(name="p1", bufs=1))

    # ---------------- DMA schedule ----------------------------------------
    # gpsimd queue: tiny consts first (pooling needs them), then kx g2,g3,
    #   then bias broadcasts.  sync queue: kx g0,g1, later cc staging.
    # scalar queue: vx 0..3, qT, then weights Wq,Wk,Wv,Wo (needed later).
    onesk_t = consts.tile([128, 64], f8e4, tag="onesk")
    onesv_t = consts.tile([128, 32], bf16, tag="onesv")
    onesf_t = consts.tile([128, 32], bf16, tag="onesf")
    onesb_t = consts.tile([32, 64], bf16, tag="onesb")
    ident_t = consts.tile([128, 128], bf16, tag="ident")
    bkT2 = consts.tile([128, 8], f32, tag="bkT2")
    nc.gpsimd.dma_start(out=onesk_t[:], in_=onesk_d[:, :])
    nc.gpsimd.dma_start(out=onesv_t[:], in_=onesv_d[:, :])
    nc.gpsimd.dma_start(out=onesf_t[:], in_=onesf_d[:, :])
    nc.gpsimd.dma_start(out=onesb_t[:], in_=onesb_d[:, :])
    nc.gpsimd.dma_start(out=ident_t[:], in_=ident_d[:, :])
    nc.gpsimd.dma_start(out=bkT2[:], in_=bkT2_d[:, :])

    kxr = kx.ap().rearrange("(g p) f -> g p f", p=128)
    vxr = vx.ap().rearrange("(g p) f -> g p f", p=128)
    kxp_cm = tc.tile_pool(name="kxp", bufs=4)
    kxp = popen(kxp_cm)
    vxp_cm = tc.tile_pool(name="vxp", bufs=2)
    vxp = popen(vxp_cm)
    qT_sb = consts.tile([128, 256], bf16, tag="qT_sb")
    nc.scalar.dma_start(out=qT_sb[:], in_=qT[:, :])
    wsb = {}
    for wname in ("WqT", "WkT", "WvT", "WoT"):
        wsb[wname] = wt.tile([128, 8192], bf16, tag=wname, name=wname)
    nc.scalar.dma_start(out=wsb["WqT"][:], in_=W["WqT"][:, :])
    kts = []
    for g in range(4):
        kt = kxp.tile([128, 16384], f8e4, tag="kt", name=f"kt{g}")
        eng = nc.sync if g % 2 == 0 else nc.gpsimd
        eng.dma_start(out=kt[:], in_=kxr[g])
        kts.append(kt)
    vts = []
    for g in range(4):
        vt = vxp.tile([128, 4096], bf16, tag="vt", name=f"vt{g}")
        nc.scalar.dma_start(out=vt[:], in_=vxr[g])
        vts.append(vt)
    for wname in ("WkT", "WvT", "WoT"):
        nc.scalar.dma_start(out=wsb[wname][:], in_=W[wname][:, :])
    bq_bc = consts.tile([32, 1024], f32, tag="bq_bc")
    bv_bc = consts.tile([64, 1024], f32, tag="bv_bc")
    bo_bc = consts.tile([32, 1024], f32, tag="bo_bc")
    bcast(bq_bc[:], bq, 32)
    bcast(bv_bc[:], bv, 64)
    bcast(bo_bc[:], bo, 32)

    # ---------------- phase D: q projection + transpose (overlaps CC) ----
    pq_cm = tc.tile_pool(name="pq", bufs=1, space="PSUM")
    pq = popen(pq_cm)
    ps_q = pq.tile([32, 1024], f32, tag="ps_q")
    for c in range(8):
        for ch in range(2):
            nc.tensor.matmul(ps_q[:, 512 * ch:512 * (ch + 1)],
                             qT_sb[:, 32 * c:32 * (c + 1)],
                             wsb["WqT"][:, 1024 * c + 512 * ch:
                                        1024 * c + 512 * (ch + 1)],
                             start=(c == 0), stop=(c == 7))
    _q_sb = p1.tile([32, 1024], bf16, tag="_q_sb")
    nc.vector.tensor_add(_q_sb[:], ps_q[:], bq_bc[:])
    # qTt3 [128, 512]: col n*32+q, head n at rows 64*(n%2); other half zero
    qTt3 = keep.tile([128, 512], bf16, tag="qTt3")
    nc.vector.memset(qTt3[:], 0.0)
    qt_ps = pq.tile([128, 512], bf16, tag="qt_ps")
    for n in range(16):
        r0 = 64 * (n % 2)
        nc.tensor.transpose(qt_ps[r0:r0 + 64, 32 * n:32 * (n + 1)],
                            _q_sb[:, 64 * n:64 * (n + 1)], ident_t[:32, :32])
    for n in range(16):
        r0 = 64 * (n % 2)
        nc.vector.tensor_copy(out=qTt3[r0:r0 + 64, 32 * n:32 * (n + 1)],
                              in_=qt_ps[r0:r0 + 64, 32 * n:32 * (n + 1)])
    pclose(pq_cm)

    # ---------------- phase B: k mean-pool (PE DoubleRow, per-level) -----
    kpool_bf = keep.tile([32, 4096], bf16, tag="kpool_bf")   # (h, l*1024+d)
    ppk_cm = tc.tile_pool(name="ppk", bufs=4, space="PSUM")
    ppk = popen(ppk_cm)
    lhs_k = onesk_t[:].rearrange("p (i m) -> p i m", i=2)
    kt4 = [kts[g][:].rearrange("p (hf l i f) -> p hf l i f", hf=2, l=4, i=2)
           for g in range(4)]
    for l in range(4):
        kp_ps = ppk.tile([32, 1024], f32, tag="kp_ps", name=f"kp_ps{l}")
        for g in range(4):
            for hf in range(2):
                for bk_ in range(2):
                    nc.tensor.matmul(
                        kp_ps[:, 512 * bk_:512 * (bk_ + 1)], lhs_k,
                        kt4[g][:, hf, l, :, 512 * bk_:512 * (bk_ + 1)],
                        start=(g == 0 and hf == 0),
                        stop=(g == 3 and hf == 1), perf_mode=DR)
        if l % 2 == 0:
            nc.vector.tensor_copy(out=kpool_bf[:, 1024 * l:1024 * (l + 1)],
                                  in_=kp_ps[:])
        else:
            nc.scalar.copy(out=kpool_bf[:, 1024 * l:1024 * (l + 1)],
                           in_=kp_ps[:])
    pclose(ppk_cm)

    # ---------------- phase C: v sum-pool (DVE adds + PE fold) -----------
    vpool_bf = keep.tile([32, 1024], bf16, tag="vpool_bf")   # sum_s v[l=3]
    vacc = p1.tile([128, 1024], f32, tag="vacc")
    for g in range(4):
        for j in range(4):
            if j >= 4 or 1024 * (j + 1) > 4096:
                continue
            sl = vts[g][:, 1024 * j:1024 * (j + 1)]
            if g == 0 and j == 0:
                nc.vector.tensor_copy(out=vacc[:], in_=sl)
            else:
                nc.vector.tensor_add(vacc[:], vacc[:], sl)
    vacc_bf = p1.tile([128, 1024], bf16, tag="vacc_bf")
    nc.vector.tensor_copy(out=vacc_bf[:], in_=vacc[:])
    ppv_cm = tc.tile_pool(name="ppv", bufs=1, space="PSUM")
    ppv = popen(ppv_cm)
    vp_ps = ppv.tile([32, 1024], f32, tag="vp_ps")
    for ch in range(2):
        nc.tensor.matmul(vp_ps[:, 512 * ch:512 * (ch + 1)], onesv_t[:],
                         vacc_bf[:, 512 * ch:512 * (ch + 1)],
                         start=True, stop=True)
    nc.vector.tensor_copy(out=vpool_bf[:], in_=vp_ps[:])
    pclose(ppv_cm)
    pclose(vxp_cm)
    pclose(kxp_cm)

    if stage < 2:
        dbg_out(vpool_bf[:])
        return

    # ---------------- phase E: pair AllGather of pooled tensors ----------
    cc_in = dram.tile([32, 5120], bf16, tag="cc_in")
    cc_out = dram.tile([64, 5120], bf16, tag="cc_out")
    nc.sync.dma_start(out=cc_in[:, :4096], in_=kpool_bf[:])
    nc.sync.dma_start(out=cc_in[:, 4096:], in_=vpool_bf[:])
    nc.gpsimd.collective_compute(
        "AllGather", mybir.AluOpType.bypass,
        replica_groups=[[0, 1], [2, 3], [4, 5], [6, 7]],
        ins=[cc_in[:].opt()], outs=[cc_out[:].opt()])

    # ---------------- phase F: gather-back + kpT/vpT transposes ----------
    kpall = p1.tile([64, 4096], bf16, tag="kpall")
    vpall = p1.tile([64, 1024], bf16, tag="vpall")
    nc.sync.dma_start(out=kpall[:], in_=cc_out[:, :4096])
    nc.sync.dma_start(out=vpall[:], in_=cc_out[:, 4096:])

    kpT = keep.tile([128, 2048], bf16, tag="kpT")   # [dd, c*256 + l*64 + h]
    vpT = keep.tile([128, 512], bf16, tag="vpT")    # [dd, c*64 + h]
    pt_cm = tc.tile_pool(name="pt", bufs=2, space="PSUM")
    pt = popen(pt_cm)
    for c in range(8):
        tp = pt.tile([128, 256], bf16, tag="ptr")
        for l in range(4):
            nc.tensor.transpose(tp[:, 64 * l:64 * (l + 1)],
                                kpall[:, 1024 * l + 128 * c:
                                      1024 * l + 128 * (c + 1)],
                                ident_t[:64, :64])
        nc.vector.tensor_copy(out=kpT[:, 256 * c:256 * (c + 1)], in_=tp[:])
    for c in range(8):
        tpv = pt.tile([128, 64], bf16, tag="ptrv")
        nc.tensor.transpose(tpv[:], vpall[:, 128 * c:128 * (c + 1)],
                            ident_t[:64, :64])
        nc.vector.tensor_copy(out=vpT[:, 64 * c:64 * (c + 1)], in_=tpv[:])

    if stage < 3:
        dbg_out(kpT[:32, :1024])
        return

    # ---------------- phase G: kbT3 = (kp @ WkT).T + bk  (bf16) ----------
    # kbT3 [128, 4096]: col n*256 + l*64 + h, head n at rows 64*(n%2),
    # other 64 rows zero (scores contract full 128 partitions from base 0).
    kbT3 = keep.tile([128, 4096], bf16, tag="kbT3")
    nc.vector.memset(kbT3[:], 0.0)
    pwg_cm = tc.tile_pool(name="pwg", bufs=2, space="PSUM")
    pwg = popen(pwg_cm)
    for jj in range(8):
        ps = pwg.tile([128, 256], f32, tag="pws")
        for c in range(8):
            nc.tensor.matmul(ps[:],
                             wsb["WkT"][:, 1024 * c + 128 * jj:
                                        1024 * c + 128 * (jj + 1)],
                             kpT[:, 256 * c:256 * (c + 1)],
                             start=(c == 0), stop=(c == 7))
        for ip in range(2):
            n = 2 * jj + ip
            r0 = 64 * ip
            nc.vector.tensor_scalar_add(
                out=kbT3[r0:r0 + 64, 256 * n:256 * (n + 1)],
                in0=ps[r0:r0 + 64, :], scalar1=bkT2[r0:r0 + 64, jj:jj + 1])
    pclose(pwg_cm)

    if stage < 31:
        dbg_out(kbT3[:32, :1024])
        return

    # ---------------- phase H: v_blk = vp @ WvT + 64*bv  [h, o] bf16 -----
    pwh_cm = tc.tile_pool(name="pwh", bufs=1, space="PSUM")
    pwh = popen(pwh_cm)
    vb_sb = keep.tile([64, 1024], bf16, tag="vb_sb")
    ps_vb = pwh.tile([64, 1024], f32, tag="ps_vb")
    for c in range(8):
        for ch in range(2):
            nc.tensor.matmul(ps_vb[:, 512 * ch:512 * (ch + 1)],
                             vpT[:, 64 * c:64 * (c + 1)],
                             wsb["WvT"][:, 1024 * c + 512 * ch:
                                        1024 * c + 512 * (ch + 1)],
                             start=(c == 0), stop=(c == 7))
    nc.vector.scalar_tensor_tensor(out=vb_sb[:], in0=bv_bc[:], scalar=64.0,
                                   in1=ps_vb[:], op0=OP.mult, op1=OP.add)
    pclose(pwh_cm)

    if stage < 32:
        dbg_out(vb_sb[:32, :])
        return

    # ---------------- phase I: scores fp32 ------------------------------
    # layout [64, 2048]: partition p = 32*(l%2) + q, col (l//2)*1024 + n*64 + h
    p2 = popen(tc.tile_pool(name="p2", bufs=1))
    st = popen(tc.tile_pool(name="stats", bufs=1))
    pwi_cm = tc.tile_pool(name="pwi", bufs=1, space="PSUM")
    pwi = popen(pwi_cm)
    scps = pwi.tile([64, 2048], f32, tag="scps")
    for l in range(4):
        p0, c0 = 32 * (l % 2), 1024 * (l // 2)
        for n in range(16):
            nc.tensor.matmul(scps[p0:p0 + 32, c0 + 64 * n:c0 + 64 * (n + 1)],
                             qTt3[:, 32 * n:32 * (n + 1)],
                             kbT3[:, 256 * n + 64 * l:256 * n + 64 * (l + 1)],
                             start=True, stop=True)
    if stage < 4:
        dbg_out(scores[:32, :1024])
        return

    # ---------------- phase J: softmax + entropy + fusion ----------------
    # p = 32*lp + q, col cb*1024 + n*64 + h, l = 2*cb + lp
    # scores stay in PSUM (scps, pre-scale); P = exp(0.125*scps) via ACT,
    # sP = (0.125*scps)*P via GpSimd -- no SBUF scores copy at all.
    P_sb = p2.tile([64, 2048], f32, tag="P_sb")
    nc.scalar.activation(out=P_sb[:], in_=scps[:], func=AF.Exp, scale=0.125)
    sP = p2.tile([64, 2048], f32, tag="sP")
    nc.vector.scalar_tensor_tensor(out=sP[:], in0=scps[:], scalar=0.125,
                                   in1=P_sb[:], op0=OP.mult, op1=OP.mult)
    Z = st.tile([64, 32], f32, tag="Z")
    nc.vector.reduce_sum(Z[:], P_sb[:].rearrange("p (m h) -> p m h", h=64), AX.X)
    S2 = st.tile([64, 32], f32, tag="S2")
    nc.vector.reduce_sum(S2[:], sP[:].rearrange("p (m h) -> p m h", h=64), AX.X)
    rZ = st.tile([64, 32], f32, tag="rZ")
    nc.vector.reciprocal(rZ[:], Z[:])
    logZ = st.tile([64, 32], f32, tag="logZ")
    nc.scalar.activation(out=logZ[:], in_=Z[:], func=AF.Ln)
    pclose(pwi_cm)

    if stage < 4:
        dbg_out(P_sb[:32, :1024])
        return

    Hent = st.tile([64, 32], f32, tag="Hent")   # entropy per p x (cb, n)
    nc.vector.tensor_mul(Hent[:], S2[:], rZ[:])
    nc.vector.tensor_sub(Hent[:], logZ[:], Hent[:])
    Hsum = st.tile([64, 2], f32, tag="Hsum")    # sum_n -> per (lp,q) x cb
    nc.vector.reduce_sum(Hsum[:], Hent[:].rearrange("p (m n) -> p m n", n=16),
                         AX.X)
    # w_lvl = softmax over l = (2*cb + lp): partial sums via PE pair-folds
    eH = st.tile([64, 2], f32, tag="eH")
    nc.scalar.activation(out=eH[:], in_=Hsum[:], func=AF.Exp,
                         scale=-1.0 / (16.0 * float(np.log(64.0))))
    E1 = st.tile([64, 1], bf16, tag="E1")       # sum over cb
    nc.vector.tensor_add(E1[:], eH[:, 0:1], eH[:, 1:2])
    pj_cm = tc.tile_pool(name="pj", bufs=1, space="PSUM")
    pj = popen(pj_cm)
    E2_ps = pj.tile([32, 1], f32, tag="E2_ps")  # sum over lp (partition pairs)
    nc.tensor.matmul(E2_ps[:], onesf_t[:64, :], E1[:], start=True, stop=True)
    E2_sb = st.tile([32, 1], bf16, tag="E2_sb")
    nc.vector.tensor_copy(out=E2_sb[:], in_=E2_ps[:])
    Eb_ps = pj.tile([64, 1], f32, tag="Eb_ps")  # broadcast back to both lp
    nc.tensor.matmul(Eb_ps[:], onesb_t[:], E2_sb[:], start=True, stop=True)
    rE = st.tile([64, 1], f32, tag="rE")
    nc.vector.reciprocal(rE[:], Eb_ps[:])
    wl = st.tile([64, 2], f32, tag="wl")        # softmax weight for l=2cb+lp
    nc.vector.tensor_scalar_mul(out=wl[:], in0=eH[:], scalar1=rE[:, :1])
    # g[p, (cb,n)] = wl[p, cb] * rZ[p, (cb,n)]; P~ = P * g (bf16)
    g = st.tile([64, 32], f32, tag="g")
    wl_ap = wl[:]
    wl_b = bass.AP(tensor=wl_ap.tensor, offset=wl_ap.offset,
                   ap=[wl_ap.ap[0], wl_ap.ap[1], [0, 16]])
    nc.vector.tensor_mul(g[:].rearrange("p (m n) -> p m n", n=16),
                         rZ[:].rearrange("p (m n) -> p m n", n=16), wl_b)
    Pt_bf = p2.tile([64, 2048], bf16, tag="Pt_bf")
    g_ap = g[:]
    for eng, half in ((nc.vector, 0), (nc.gpsimd, 1)):
        g_h = bass.AP(tensor=g_ap.tensor, offset=g_ap.offset + 16 * half,
                      ap=[g_ap.ap[0], [g_ap.ap[1][0], 16], [0, 64]])
        eng.tensor_mul(
            Pt_bf[:, 1024 * half:1024 * (half + 1)].rearrange(
                "p (m h) -> p m h", h=64),
            P_sb[:, 1024 * half:1024 * (half + 1)].rearrange(
                "p (m h) -> p m h", h=64), g_h)
    # fused[q, (n,h)] = sum over lp (pair-fold) and cb (psum accumulate)
    fused_ps = pj.tile([32, 1024], f32, tag="fused_ps")
    for cb in range(2):
        for ch in range(2):
            nc.tensor.matmul(fused_ps[:, 512 * ch:512 * (ch + 1)],
                             onesf_t[:64, :],
                             Pt_bf[:, 1024 * cb + 512 * ch:
                                   1024 * cb + 512 * (ch + 1)],
                             start=(cb == 0), stop=(cb == 1))
    fused_bf = st.tile([32, 1024], bf16, tag="fused_bf")
    nc.vector.tensor_copy(out=fused_bf[:], in_=fused_ps[:])
    pclose(pj_cm)

    # ---------------- phase K: ctx = fused @ v_blk ----------------------
    pwk_cm = tc.tile_pool(name="pwk", bufs=1, space="PSUM")
    pwk = popen(pwk_cm)
    fusedT = st.tile([64, 512], bf16, tag="fusedT")  # col n*32 + q
    ps_ft = pwk.tile([64, 512], bf16, tag="ps_ft")
    for n in range(16):
        nc.tensor.transpose(ps_ft[:, 32 * n:32 * (n + 1)],
                            fused_bf[:, 64 * n:64 * (n + 1)], ident_t[:32, :32])
    nc.vector.tensor_copy(out=fusedT[:], in_=ps_ft[:])
    ps_ctx = pwk.tile([32, 1024], f32, tag="ps_ctx")
    for n in range(16):
        nc.tensor.matmul(ps_ctx[:, 64 * n:64 * (n + 1)],
                         fusedT[:, 32 * n:32 * (n + 1)],
                         vb_sb[:, 64 * n:64 * (n + 1)], start=True, stop=True)
    ctx_sb = st.tile([32, 1024], bf16, tag="ctx_sb")
    for cg in range(4):
        nc.vector.tensor_copy(out=ctx_sb[:, 256 * cg:256 * (cg + 1)],
                              in_=ps_ctx[:, 256 * cg:256 * (cg + 1)])
    pclose(pwk_cm)

    # ---------------- phase L: faithful-reshape scramble + o_proj --------
    # Y[r=(2n+jp), qq*64+dh] = ctx[16*jp+qq, 64*n+dh]; spread over 3 queues
    Y_sb = st.tile([32, 1024], bf16, tag="Y_sb")
    for n in range(16):
        eng = (nc.gpsimd, nc.sync, nc.scalar)[n % 3]
        eng.dma_start(out=Y_sb[2 * n:2 * n + 2, :],
                      in_=ctx_sb[:, 64 * n:64 * (n + 1)])
    pwl_cm = tc.tile_pool(name="pwl", bufs=1, space="PSUM")
    pwl = popen(pwl_cm)
    YT = st.tile([128, 256], bf16, tag="YT")  # col mc*32+r, row mm
    for mg in range(2):
        tp = pt.tile([128, 256], bf16, tag="ptr")
        for j in range(4):
            mc = 4 * mg + j
            nc.tensor.transpose(tp[:, 32 * j:32 * (j + 1)],
                                Y_sb[:, 128 * mc:128 * (mc + 1)], ident_t[:32, :32])
        nc.vector.tensor_copy(out=YT[:, 128 * mg:128 * (mg + 1)], in_=tp[:, :128])

    out_sb = st.tile([32, 1024], f32, tag="out_sb")
    ps_o = pwl.tile([32, 1024], f32, tag="ps_o")
    for ch in range(2):
        for mc in range(8):
            nc.tensor.matmul(ps_o[:, 512 * ch:512 * (ch + 1)],
                             YT[:, 32 * mc:32 * (mc + 1)],
                             wsb["WoT"][:, 1024 * mc + 512 * ch:
                                        1024 * mc + 512 * (ch + 1)],
                             start=(mc == 0), stop=(mc == 7))
    nc.vector.tensor_add(out_sb[:], ps_o[:], bo_bc[:])
    nc.sync.dma_start(out=out_d[:, :], in_=out_sb[:])

    finish()


def _get_nc():
    if "nc" not in _CACHE:
        _CACHE["nc"] = _build_nc()
    return _CACHE["nc"]


def make_in_maps(q, k, v, Wq, bq, Wk, bk, Wv, bv, Wo, bo):
    import ml_dtypes
    f8 = ml_dtypes.float8_e4m3
    bf = ml_dtypes.bfloat16

    q = np.asarray(q, np.float32)
    k = np.asarray(k, np.float32)
    v = np.asarray(v, np.float32)
    Ws = {"Wq": np.asarray(Wq, np.float32), "Wk": np.asarray(Wk, np.float32),
          "Wv": np.asarray(Wv, np.float32), "Wo": np.asarray(Wo, np.float32)}
    bs = {"bq": np.asarray(bq, np.float32), "bk": np.asarray(bk, np.float32),
          "bv": np.asarray(bv, np.float32), "bo": np.asarray(bo, np.float32)}

    # shared across cores
    WT = {}
    for n_, w in Ws.items():
        WT[n_ + "T"] = np.ascontiguousarray(
            w.T.reshape(8, 128, 1024).transpose(1, 0, 2).reshape(128, 8192)
        ).astype(bf)
    bkT2 = np.ascontiguousarray(bs["bk"].reshape(8, 128).T)  # [128, 8]
    pidx = np.arange(128)
    onesk = np.zeros((128, 64), np.float32)
    for i in range(2):
        onesk[pidx, i * 32 + pidx // 4] = 1.0 / 64.0
    onesk = onesk.astype(f8)
    onesv = np.zeros((128, 32), np.float32)
    onesv[pidx, pidx // 4] = 1.0
    onesv = onesv.astype(bf)
    onesf = np.zeros((128, 32), np.float32)
    onesf[pidx, pidx % 32] = 1.0
    onesf = onesf.astype(bf)
    onesb = np.ascontiguousarray(onesf[:64].T).astype(bf)
    ident = np.eye(128, dtype=np.float32).astype(bf)

    in_maps = []
    for c in range(NCORES):
        b, half = c // 2, c % 2
        hs = slice(32 * half, 32 * half + 32)
        # kx rows g*128 + (4h+sa), cols hf*8192 + l*2048 + i*1024 + d,
        # with s = (2g+hf)*8 + sa*2 + i
        kc = k[b, hs]                                  # (32, 64, 4, 1024)
        kc = kc.reshape(32, 4, 2, 4, 2, 4, 1024)       # h, g, hf, sa, i, l, d
        kc = kc.transpose(1, 0, 3, 2, 5, 4, 6).reshape(512, 16384)
        # vx rows g*128 + (4h+sa), cols j*1024 + d, s = (g*4+j)*4 + sa
        vc = v[b, hs, :, L - 1, :]                     # (32, 64, 1024)
        vc = vc.reshape(32, 4, 4, 4, 1024)             # h, g, j, sa, d
        vc = vc.transpose(1, 0, 3, 2, 4).reshape(512, 4096)
        # qT: [dd, c*32 + r]
        qc = q[b, hs]                                  # (32, 1024)
        qTc = qc.T.reshape(8, 128, 32).transpose(1, 0, 2).reshape(128, 256)
        in_maps.append(dict(
            kx=np.ascontiguousarray(kc).astype(f8),
            vx=np.ascontiguousarray(vc).astype(bf),
            qT=np.ascontiguousarray(qTc).astype(bf),
            **WT, bq=bs["bq"], bkT2=bkT2, bv=bs["bv"], bo=bs["bo"],
            onesk=onesk, onesv=onesv, onesf=onesf, onesb=onesb,
            ident=ident))
    return in_maps


def assemble(results):
    out = np.empty((B, H, D), np.float32)
    for c in range(NCORES):
        b, half = c // 2, c % 2
        o = results[c]["out"]  # rows r = 2n + jp  ->  h' = 4n + 2*half + jp
        for r in range(32):
            out[b, 4 * (r // 2) + 2 * half + (r % 2)] = o[r]
    return out


def _install_ntff_shim():
    """Register the axon NTFF profile hook if the image's antenv lacks it."""
    import sys
    import types
    try:
        if "antenv.axon_hooks" in sys.modules:
            return
        import antenv
        mod = types.ModuleType("antenv.axon_hooks")
        mod._hook = None

        def set_axon_ntff_profile_hook(h):
            mod._hook = h

        def get_axon_ntff_profile_hook():
            return mod._hook

        mod.set_axon_ntff_profile_hook = set_axon_ntff_profile_hook
        mod.get_axon_ntff_profile_hook = get_axon_ntff_profile_hook
        sys.modules["antenv.axon_hooks"] = mod
        antenv.axon_hooks = mod
        from trn_agent_boot.trn_boot import _ntff_profile_via_ctypes
        hook = _ntff_profile_via_ctypes("/opt/axon/libaxon_pjrt.so")
        if hook is not None:
            set_axon_ntff_profile_hook(hook)
    except Exception:
        pass  # tracing degrades; execution unaffected


def kernel(q, k, v, Wq, bq, Wk, bk, Wv, bv, Wo, bo, _trace=False):
    global LAST_RESULTS
    from concourse.bass_utils import run_bass_kernel_spmd
    if _trace:
        _install_ntff_shim()
    nc = _get_nc()
    in_maps = make_in_maps(q, k, v, Wq, bq, Wk, bk, Wv, bv, Wo, bo)
    res = run_bass_kernel_spmd(nc, in_maps, list(range(NCORES)), trace=_trace)
    LAST_RESULTS = res
    return assemble(res.results)


# revision 43
# speedup vs baseline: 1.1107x; 1.0052x over previous
"""Trainium2 Bass kernel for nn_BlockCrossAttention (B=4,H=64,S=64,L=4,D=1024,NH=16).

Sharding: core c in 0..7 -> (b = c//2, half = c%2); 32 query/head rows per core.
Host prep (not counted in HW time): dtype casts + layout packs only --
  kx fp8-e4m3 in DoubleRow pooling layout (4 x 2MB chunks, 16KB lines),
  vx bf16 (4 x 1MB chunks), weights bf16 pre-transposed/chunked, qT
  pre-arranged, block-diag "ones" reducers.
On-chip: k mean-pool via PE DoubleRow matmuls accumulating in PSUM (fp32),
v sum-pool via DVE adds + PE fold, pair AllGather of pooled tensors (bf16),
bf16 projections/attention, softmax+entropy stats on full 128 partitions
(scores laid out [(l,q), (n,h)]), entropy-gated fusion via PE fold,
faithful-reshape scramble, o_proj. Output rows disjoint across cores.
DMA spread over sync/gpsimd/scalar queues: big inputs first, weights behind.
"""

import numpy as np

B, H, S, L, D = 4, 64, 64, 4, 1024
NH, DH = 16, 64
NCORES = 8

_CACHE = {}
LAST_RESULTS = None  # test.py reads exec_time from here


def _build_nc(stage=99):
    import concourse.bacc as bacc
    import concourse.bass as bass
    import concourse.tile as tile
    from concourse import mybir
    from concourse.masks import make_identity

    f32 = mybir.dt.float32
    bf16 = mybir.dt.bfloat16
    f8e4 = mybir.dt.float8e4
    AF = mybir.ActivationFunctionType
    AX = mybir.AxisListType
    OP = mybir.AluOpType
    DR = mybir.MatmulPerfMode.DoubleRow

    nc = bacc.Bacc("TRN2", target_bir_lowering=False, debug=False, num_devices=NCORES)

    kx = nc.dram_tensor("kx", [512, 16384], f8e4, kind="ExternalInput")
    vx = nc.dram_tensor("vx", [512, 4096], bf16, kind="ExternalInput")
    qT = nc.dram_tensor("qT", [128, 256], bf16, kind="ExternalInput")
    W = {w: nc.dram_tensor(w, [128, 8192], bf16, kind="ExternalInput")
         for w in ("WqT", "WkT", "WvT", "WoT")}
    bq = nc.dram_tensor("bq", [1024], f32, kind="ExternalInput")
    bkT2 = nc.dram_tensor("bkT2", [128, 8], f32, kind="ExternalInput")
    bv = nc.dram_tensor("bv", [1024], f32, kind="ExternalInput")
    bo = nc.dram_tensor("bo", [1024], f32, kind="ExternalInput")
    onesk = nc.dram_tensor("onesk", [128, 64], f8e4, kind="ExternalInput")
    onesv = nc.dram_tensor("onesv", [128, 32], bf16, kind="ExternalInput")
    onesf = nc.dram_tensor("onesf", [128, 32], bf16, kind="ExternalInput")
    onesb = nc.dram_tensor("onesb", [32, 64], bf16, kind="ExternalInput")
    ident = nc.dram_tensor("ident", [128, 128], bf16, kind="ExternalInput")
    out_d = nc.dram_tensor("out", [32, 1024], f32, kind="ExternalOutput")
    cc_out_sh = nc.dram_tensor("cc_out_sh", [64, 5120], bf16, kind="Internal",
                               addr_space="Shared")

    with tile.TileContext(nc) as tc:
        _emit(nc, tc, bass, mybir, f32, bf16, f8e4, AF, AX, OP, DR,
              make_identity, kx, vx, qT, W, bq, bkT2, bv, bo,
              onesk, onesv, onesf, onesb, ident, out_d, cc_out_sh, stage)
    nc.compile()
    return nc


def _emit(nc, tc, bass, mybir, f32, bf16, f8e4, AF, AX, OP, DR, make_identity,
          kx, vx, qT, W, bq, bkT2_d, bv, bo, onesk_d, onesv_d, onesf_d,
          onesb_d, ident_d, out_d, cc_out_sh, stage=99):
    stack = []

    def popen(cm):
        stack.append(cm)
        return cm.__enter__()

    def pclose(cm):
        assert stack and stack[-1] is cm, "pool close order"
        stack.pop()
        cm.__exit__(None, None, None)

    def finish():
        for cm in reversed(stack[:]):
            pclose(cm)

    def bcast(dst_ap, src_t, n):
        ap = src_t.ap()
        nc.gpsimd.dma_start(out=dst_ap, in_=bass.AP(
            tensor=ap.tensor, offset=ap.offset, ap=[[0, n]] + list(ap.ap)))

    def dbg_out(src_ap):
        dbg_cm = popen(tc.tile_pool(name="dbg", bufs=1))
        dbg = dbg_cm.tile([32, 1024], f32, tag="dbg")
        nc.vector.tensor_copy(out=dbg[:], in_=src_ap)
        nc.sync.dma_start(out=out_d[:, :], in_=dbg[:])
        finish()

    consts = popen(tc.tile_pool(name="consts", bufs=1))
    keep = popen(tc.tile_pool(name="keep", bufs=1))
    wt = popen(tc.tile_pool(name="wt", bufs=1))
    dram = popen(tc.tile_pool(name="dram", bufs=1, space="DRAM"))
    p1 = popen(tc.tile_pool(name="p1", bufs=1))

    # ---------------- DMA schedule ----------------------------------------
    # gpsimd queue: tiny consts first (pooling needs them), then kx g2,g3,
    #   then bias broadcasts.  sync queue: kx g0,g1, later cc staging.
    # scalar queue: vx 0..3, qT, then weights Wq,Wk,Wv,Wo (needed later).
    onesk_t = consts.tile([128, 64], f8e4, tag="onesk")
    onesv_t = consts.tile([128, 32], bf16, tag="onesv")
    onesf_t = consts.tile([128, 32], bf16, tag="onesf")
    onesb_t = consts.tile([32, 64], bf16, tag="onesb")
    ident_t = consts.tile([128, 128], bf16, tag="ident")
    bkT2 = consts.tile([128, 8], f32, tag="bkT2")
    nc.gpsimd.dma_start(out=onesk_t[:], in_=onesk_d[:, :])
    nc.gpsimd.dma_start(out=onesv_t[:], in_=onesv_d[:, :])
    nc.gpsimd.dma_start(out=onesf_t[:], in_=onesf_d[:, :])
    nc.gpsimd.dma_start(out=onesb_t[:], in_=onesb_d[:, :])
    nc.gpsimd.dma_start(out=ident_t[:], in_=ident_d[:, :])
    nc.gpsimd.dma_start(out=bkT2[:], in_=bkT2_d[:, :])

    kxr = kx.ap().rearrange("(g p) f -> g p f", p=128)
    vxr = vx.ap().rearrange("(g p) f -> g p f", p=128)
    kxp_cm = tc.tile_pool(name="kxp", bufs=4)
    kxp = popen(kxp_cm)
    vxp_cm = tc.tile_pool(name="vxp", bufs=2)
    vxp = popen(vxp_cm)
    qT_sb = consts.tile([128, 256], bf16, tag="qT_sb")
    nc.scalar.dma_start(out=qT_sb[:], in_=qT[:, :])
    wsb = {}
    for wname in ("WqT", "WkT", "WvT", "WoT"):
        wsb[wname] = wt.tile([128, 8192], bf16, tag=wname, name=wname)
    nc.scalar.dma_start(out=wsb["WqT"][:], in_=W["WqT"][:, :])
    kts = []
    for g in range(4):
        kt = kxp.tile([128, 16384], f8e4, tag="kt", name=f"kt{g}")
        eng = nc.sync if g % 2 == 0 else nc.gpsimd
        eng.dma_start(out=kt[:], in_=kxr[g])
        kts.append(kt)
    vts = []
    for g in range(4):
        vt = vxp.tile([128, 4096], bf16, tag="vt", name=f"vt{g}")
        nc.scalar.dma_start(out=vt[:], in_=vxr[g])
        vts.append(vt)
    for wname in ("WkT", "WvT", "WoT"):
        nc.scalar.dma_start(out=wsb[wname][:], in_=W[wname][:, :])
    bq_bc = consts.tile([32, 1024], f32, tag="bq_bc")
    bv_bc = consts.tile([64, 1024], f32, tag="bv_bc")
    bo_bc = consts.tile([32, 1024], f32, tag="bo_bc")
    bcast(bq_bc[:], bq, 32)
    bcast(bv_bc[:], bv, 64)
    bcast(bo_bc[:], bo, 32)

    # ---------------- phase D: q projection + transpose (overlaps CC) ----
    pq_cm = tc.tile_pool(name="pq", bufs=1, space="PSUM")
    pq = popen(pq_cm)
    ps_q = pq.tile([32, 1024], f32, tag="ps_q")
    for c in range(8):
        for ch in range(2):
            nc.tensor.matmul(ps_q[:, 512 * ch:512 * (ch + 1)],
                             qT_sb[:, 32 * c:32 * (c + 1)],
                             wsb["WqT"][:, 1024 * c + 512 * ch:
                                        1024 * c + 512 * (ch + 1)],
                             start=(c == 0), stop=(c == 7))
    _q_sb = p1.tile([32, 1024], bf16, tag="_q_sb")
    nc.vector.tensor_add(_q_sb[:], ps_q[:], bq_bc[:])
    # qTt3 [128, 512]: col n*32+q, head n at rows 64*(n%2); other half zero
    qTt3 = keep.tile([128, 512], bf16, tag="qTt3")
    nc.vector.memset(qTt3[:], 0.0)
    qt_ps = pq.tile([128, 512], bf16, tag="qt_ps")
    for n in range(16):
        r0 = 64 * (n % 2)
        nc.tensor.transpose(qt_ps[r0:r0 + 64, 32 * n:32 * (n + 1)],
                            _q_sb[:, 64 * n:64 * (n + 1)], ident_t[:32, :32])
    for n in range(16):
        r0 = 64 * (n % 2)
        nc.vector.tensor_copy(out=qTt3[r0:r0 + 64, 32 * n:32 * (n + 1)],
                              in_=qt_ps[r0:r0 + 64, 32 * n:32 * (n + 1)])
    pclose(pq_cm)

    # ---------------- phase B: k mean-pool (PE DoubleRow, per-level) -----
    kpool_bf = keep.tile([32, 4096], bf16, tag="kpool_bf")   # (h, l*1024+d)
    ppk_cm = tc.tile_pool(name="ppk", bufs=4, space="PSUM")
    ppk = popen(ppk_cm)
    lhs_k = onesk_t[:].rearrange("p (i m) -> p i m", i=2)
    kt4 = [kts[g][:].rearrange("p (hf l i f) -> p hf l i f", hf=2, l=4, i=2)
           for g in range(4)]
    for l in range(4):
        kp_ps = ppk.tile([32, 1024], f32, tag="kp_ps", name=f"kp_ps{l}")
        for g in range(4):
            for hf in range(2):
                for bk_ in range(2):
                    nc.tensor.matmul(
                        kp_ps[:, 512 * bk_:512 * (bk_ + 1)], lhs_k,
                        kt4[g][:, hf, l, :, 512 * bk_:512 * (bk_ + 1)],
                        start=(g == 0 and hf == 0),
                        stop=(g == 3 and hf == 1), perf_mode=DR)
        nc.vector.tensor_copy(out=kpool_bf[:, 1024 * l:1024 * (l + 1)],
                              in_=kp_ps[:])
    pclose(ppk_cm)

    # ---------------- phase C: v sum-pool (DVE adds + PE fold) -----------
    vpool_bf = keep.tile([32, 1024], bf16, tag="vpool_bf")   # sum_s v[l=3]
    vacc = p1.tile([128, 1024], f32, tag="vacc")
    for g in range(4):
        for j in range(4):
            if j >= 4 or 1024 * (j + 1) > 4096:
                continue
            sl = vts[g][:, 1024 * j:1024 * (j + 1)]
            if g == 0 and j == 0:
                nc.vector.tensor_copy(out=vacc[:], in_=sl)
            else:
                nc.vector.tensor_add(vacc[:], vacc[:], sl)
    vacc_bf = p1.tile([128, 1024], bf16, tag="vacc_bf")
    nc.vector.tensor_copy(out=vacc_bf[:], in_=vacc[:])
    ppv_cm = tc.tile_pool(name="ppv", bufs=1, space="PSUM")
    ppv = popen(ppv_cm)
    vp_ps = ppv.tile([32, 1024], f32, tag="vp_ps")
    for ch in range(2):
        nc.tensor.matmul(vp_ps[:, 512 * ch:512 * (ch + 1)], onesv_t[:],
                         vacc_bf[:, 512 * ch:512 * (ch + 1)],
                         start=True, stop=True)
    nc.vector.tensor_copy(out=vpool_bf[:], in_=vp_ps[:])
    pclose(ppv_cm)
    pclose(vxp_cm)
    pclose(kxp_cm)

    if stage < 2:
        dbg_out(vpool_bf[:])
        return

    # ---------------- phase E: pair AllGather of pooled tensors ----------
    cc_in = dram.tile([32, 5120], bf16, tag="cc_in")
    cc_out = dram.tile([64, 5120], bf16, tag="cc_out")
    nc.sync.dma_start(out=cc_in[:, :4096], in_=kpool_bf[:])
    nc.sync.dma_start(out=cc_in[:, 4096:], in_=vpool_bf[:])
    nc.gpsimd.collective_compute(
        "AllGather", mybir.AluOpType.bypass,
        replica_groups=[[0, 1], [2, 3], [4, 5], [6, 7]],
        ins=[cc_in[:].opt()], outs=[cc_out[:].opt()])

    # ---------------- phase F: gather-back + kpT/vpT transposes ----------
    kpall = p1.tile([64, 4096], bf16, tag="kpall")
    vpall = p1.tile([64, 1024], bf16, tag="vpall")
    nc.sync.dma_start(out=kpall[:], in_=cc_out[:, :4096])
    nc.sync.dma_start(out=vpall[:], in_=cc_out[:, 4096:])

    kpT = keep.tile([128, 2048], bf16, tag="kpT")   # [dd, c*256 + l*64 + h]
    vpT = keep.tile([128, 512], bf16, tag="vpT")    # [dd, c*64 + h]
    pt_cm = tc.tile_pool(name="pt", bufs=2, space="PSUM")
    pt = popen(pt_cm)
    for c in range(8):
        tp = pt.tile([128, 256], bf16, tag="ptr")
        for l in range(4):
            nc.tensor.transpose(tp[:, 64 * l:64 * (l + 1)],
                                kpall[:, 1024 * l + 128 * c:
                                      1024 * l + 128 * (c + 1)],
                                ident_t[:64, :64])
        nc.vector.tensor_copy(out=kpT[:, 256 * c:256 * (c + 1)], in_=tp[:])
    for c in range(8):
        tpv = pt.tile([128, 64], bf16, tag="ptrv")
        nc.tensor.transpose(tpv[:], vpall[:, 128 * c:128 * (c + 1)],
                            ident_t[:64, :64])
        nc.vector.tensor_copy(out=vpT[:, 64 * c:64 * (c + 1)], in_=tpv[:])

    if stage < 3:
        dbg_out(kpT[:32, :1024])
        return

    # ---------------- phase G: kbT3 = (kp @ WkT).T + bk  (bf16) ----------
    # kbT3 [128, 4096]: col n*256 + l*64 + h, head n at rows 64*(n%2),
    # other 64 rows zero (scores contract full 128 partitions from base 0).
    kbT3 = keep.tile([128, 4096], bf16, tag="kbT3")
    nc.vector.memset(kbT3[:], 0.0)
    pwg_cm = tc.tile_pool(name="pwg", bufs=2, space="PSUM")
    pwg = popen(pwg_cm)
    for jj in range(8):
        ps = pwg.tile([128, 256], f32, tag="pws")
        for c in range(8):
            nc.tensor.matmul(ps[:],
                             wsb["WkT"][:, 1024 * c + 128 * jj:
                                        1024 * c + 128 * (jj + 1)],
                             kpT[:, 256 * c:256 * (c + 1)],
                             start=(c == 0), stop=(c == 7))
        for ip in range(2):
            n = 2 * jj + ip
            r0 = 64 * ip
            nc.vector.tensor_scalar_add(
                out=kbT3[r0:r0 + 64, 256 * n:256 * (n + 1)],
                in0=ps[r0:r0 + 64, :], scalar1=bkT2[r0:r0 + 64, jj:jj + 1])
    pclose(pwg_cm)

    if stage < 31:
        dbg_out(kbT3[:32, :1024])
        return

    # ---------------- phase H: v_blk = vp @ WvT + 64*bv  [h, o] bf16 -----
    pwh_cm = tc.tile_pool(name="pwh", bufs=1, space="PSUM")
    pwh = popen(pwh_cm)
    vb_sb = keep.tile([64, 1024], bf16, tag="vb_sb")
    ps_vb = pwh.tile([64, 1024], f32, tag="ps_vb")
    for c in range(8):
        for ch in range(2):
            nc.tensor.matmul(ps_vb[:, 512 * ch:512 * (ch + 1)],
                             vpT[:, 64 * c:64 * (c + 1)],
                             wsb["WvT"][:, 1024 * c + 512 * ch:
                                        1024 * c + 512 * (ch + 1)],
                             start=(c == 0), stop=(c == 7))
    nc.vector.scalar_tensor_tensor(out=vb_sb[:], in0=bv_bc[:], scalar=64.0,
                                   in1=ps_vb[:], op0=OP.mult, op1=OP.add)
    pclose(pwh_cm)

    if stage < 32:
        dbg_out(vb_sb[:32, :])
        return

    # ---------------- phase I: scores fp32 ------------------------------
    # layout [64, 2048]: partition p = 32*(l%2) + q, col (l//2)*1024 + n*64 + h
    p2 = popen(tc.tile_pool(name="p2", bufs=1))
    st = popen(tc.tile_pool(name="stats", bufs=1))
    pwi_cm = tc.tile_pool(name="pwi", bufs=1, space="PSUM")
    pwi = popen(pwi_cm)
    scps = pwi.tile([64, 2048], f32, tag="scps")
    for l in range(4):
        p0, c0 = 32 * (l % 2), 1024 * (l // 2)
        for n in range(16):
            nc.tensor.matmul(scps[p0:p0 + 32, c0 + 64 * n:c0 + 64 * (n + 1)],
                             qTt3[:, 32 * n:32 * (n + 1)],
                             kbT3[:, 256 * n + 64 * l:256 * n + 64 * (l + 1)],
                             start=True, stop=True)
    if stage < 4:
        dbg_out(scores[:32, :1024])
        return

    # ---------------- phase J: softmax + entropy + fusion ----------------
    # p = 32*lp + q, col cb*1024 + n*64 + h, l = 2*cb + lp
    # scores stay in PSUM (scps, pre-scale); P = exp(0.125*scps) via ACT,
    # sP = (0.125*scps)*P via GpSimd -- no SBUF scores copy at all.
    P_sb = p2.tile([64, 2048], f32, tag="P_sb")
    nc.scalar.activation(out=P_sb[:], in_=scps[:], func=AF.Exp, scale=0.125)
    sP = p2.tile([64, 2048], f32, tag="sP")
    nc.vector.scalar_tensor_tensor(out=sP[:], in0=scps[:], scalar=0.125,
                                   in1=P_sb[:], op0=OP.mult, op1=OP.mult)
    Z = st.tile([64, 32], f32, tag="Z")
    nc.vector.reduce_sum(Z[:], P_sb[:].rearrange("p (m h) -> p m h", h=64), AX.X)
    S2 = st.tile([64, 32], f32, tag="S2")
    nc.vector.reduce_sum(S2[:], sP[:].rearrange("p (m h) -> p m h", h=64), AX.X)
    rZ = st.tile([64, 32], f32, tag="rZ")
    nc.vector.reciprocal(rZ[:], Z[:])
    logZ = st.tile([64, 32], f32, tag="logZ")
    nc.scalar.activation(out=logZ[:], in_=Z[:], func=AF.Ln)
    pclose(pwi_cm)

    if stage < 4:
        dbg_out(P_sb[:32, :1024])
        return

    Hent = st.tile([64, 32], f32, tag="Hent")   # entropy per p x (cb, n)
    nc.vector.tensor_mul(Hent[:], S2[:], rZ[:])
    nc.vector.tensor_sub(Hent[:], logZ[:], Hent[:])
    Hsum = st.tile([64, 2], f32, tag="Hsum")    # sum_n -> per (lp,q) x cb
    nc.vector.reduce_sum(Hsum[:], Hent[:].rearrange("p (m n) -> p m n", n=16),
                         AX.X)
    # w_lvl = softmax over l = (2*cb + lp): partial sums via PE pair-folds
    eH = st.tile([64, 2], f32, tag="eH")
    nc.scalar.activation(out=eH[:], in_=Hsum[:], func=AF.Exp,
                         scale=-1.0 / (16.0 * float(np.log(64.0))))
    E1 = st.tile([64, 1], bf16, tag="E1")       # sum over cb
    nc.vector.tensor_add(E1[:], eH[:, 0:1], eH[:, 1:2])
    pj_cm = tc.tile_pool(name="pj", bufs=1, space="PSUM")
    pj = popen(pj_cm)
    E2_ps = pj.tile([32, 1], f32, tag="E2_ps")  # sum over lp (partition pairs)
    nc.tensor.matmul(E2_ps[:], onesf_t[:64, :], E1[:], start=True, stop=True)
    E2_sb = st.tile([32, 1], bf16, tag="E2_sb")
    nc.vector.tensor_copy(out=E2_sb[:], in_=E2_ps[:])
    Eb_ps = pj.tile([64, 1], f32, tag="Eb_ps")  # broadcast back to both lp
    nc.tensor.matmul(Eb_ps[:], onesb_t[:], E2_sb[:], start=True, stop=True)
    rE = st.tile([64, 1], f32, tag="rE")
    nc.vector.reciprocal(rE[:], Eb_ps[:])
    wl = st.tile([64, 2], f32, tag="wl")        # softmax weight for l=2cb+lp
    nc.vector.tensor_scalar_mul(out=wl[:], in0=eH[:], scalar1=rE[:, :1])
    # g[p, (cb,n)] = wl[p, cb] * rZ[p, (cb,n)]; P~ = P * g (bf16)
    g = st.tile([64, 32], f32, tag="g")
    wl_ap = wl[:]
    wl_b = bass.AP(tensor=wl_ap.tensor, offset=wl_ap.offset,
                   ap=[wl_ap.ap[0], wl_ap.ap[1], [0, 16]])
    nc.vector.tensor_mul(g[:].rearrange("p (m n) -> p m n", n=16),
                         rZ[:].rearrange("p (m n) -> p m n", n=16), wl_b)
    Pt_bf = p2.tile([64, 2048], bf16, tag="Pt_bf")
    g_ap = g[:]
    for eng, half in ((nc.vector, 0), (nc.gpsimd, 1)):
        g_h = bass.AP(tensor=g_ap.tensor, offset=g_ap.offset + 16 * half,
                      ap=[g_ap.ap[0], [g_ap.ap[1][0], 16], [0, 64]])
        eng.tensor_mul(
            Pt_bf[:, 1024 * half:1024 * (half + 1)].rearrange(
                "p (m h) -> p m h", h=64),
            P_sb[:, 1024 * half:1024 * (half + 1)].rearrange(
                "p (m h) -> p m h", h=64), g_h)
    # fused[q, (n,h)] = sum over lp (pair-fold) and cb (psum accumulate)
    fused_ps = pj.tile([32, 1024], f32, tag="fused_ps")
    for cb in range(2):
        for ch in range(2):
            nc.tensor.matmul(fused_ps[:, 512 * ch:512 * (ch + 1)],
                             onesf_t[:64, :],
                             Pt_bf[:, 1024 * cb + 512 * ch:
                                   1024 * cb + 512 * (ch + 1)],
                             start=(cb == 0), stop=(cb == 1))
    fused_bf = st.tile([32, 1024], bf16, tag="fused_bf")
    nc.vector.tensor_copy(out=fused_bf[:], in_=fused_ps[:])
    pclose(pj_cm)

    # ---------------- phase K: ctx = fused @ v_blk ----------------------
    pwk_cm = tc.tile_pool(name="pwk", bufs=1, space="PSUM")
    pwk = popen(pwk_cm)
    fusedT = st.tile([64, 512], bf16, tag="fusedT")  # col n*32 + q
    ps_ft = pwk.tile([64, 512], bf16, tag="ps_ft")
    for n in range(16):
        nc.tensor.transpose(ps_ft[:, 32 * n:32 * (n + 1)],
                            fused_bf[:, 64 * n:64 * (n + 1)], ident_t[:32, :32])
    nc.vector.tensor_copy(out=fusedT[:], in_=ps_ft[:])
    ps_ctx = pwk.tile([32, 1024], f32, tag="ps_ctx")
    for n in range(16):
        nc.tensor.matmul(ps_ctx[:, 64 * n:64 * (n + 1)],
                         fusedT[:, 32 * n:32 * (n + 1)],
                         vb_sb[:, 64 * n:64 * (n + 1)], start=True, stop=True)
    ctx_sb = st.tile([32, 1024], bf16, tag="ctx_sb")
    for cg in range(4):
        nc.vector.tensor_copy(out=ctx_sb[:, 256 * cg:256 * (cg + 1)],
                              in_=ps_ctx[:, 256 * cg:256 * (cg + 1)])
    pclose(pwk_cm)

    # ---------------- phase L: faithful-reshape scramble + o_proj --------
    # Y[r=(2n+jp), qq*64+dh] = ctx[16*jp+qq, 64*n+dh]; spread over 3 queues
    Y_sb = st.tile([32, 1024], bf16, tag="Y_sb")
    for n in range(16):
        eng = (nc.gpsimd, nc.sync, nc.scalar)[n % 3]
        eng.dma_start(out=Y_sb[2 * n:2 * n + 2, :],
                      in_=ctx_sb[:, 64 * n:64 * (n + 1)])
    pwl_cm = tc.tile_pool(name="pwl", bufs=1, space="PSUM")
    pwl = popen(pwl_cm)
    YT = st.tile([128, 256], bf16, tag="YT")  # col mc*32+r, row mm
    for mg in range(2):
        tp = pt.tile([128, 256], bf16, tag="ptr")
        for j in range(4):
            mc = 4 * mg + j
            nc.tensor.transpose(tp[:, 32 * j:32 * (j + 1)],
                                Y_sb[:, 128 * mc:128 * (mc + 1)], ident_t[:32, :32])
        nc.vector.tensor_copy(out=YT[:, 128 * mg:128 * (mg + 1)], in_=tp[:, :128])

    out_sb = st.tile([32, 1024], f32, tag="out_sb")
    ps_o = pwl.tile([32, 1024], f32, tag="ps_o")
    for ch in range(2):
        for mc in range(8):
            nc.tensor.matmul(ps_o[:, 512 * ch:512 * (ch + 1)],
                             YT[:, 32 * mc:32 * (mc + 1)],
                             wsb["WoT"][:, 1024 * mc + 512 * ch:
                                        1024 * mc + 512 * (ch + 1)],
                             start=(mc == 0), stop=(mc == 7))
    nc.vector.tensor_add(out_sb[:], ps_o[:], bo_bc[:])
    nc.sync.dma_start(out=out_d[:, :], in_=out_sb[:])

    finish()


def _get_nc():
    if "nc" not in _CACHE:
        _CACHE["nc"] = _build_nc()
    return _CACHE["nc"]


def make_in_maps(q, k, v, Wq, bq, Wk, bk, Wv, bv, Wo, bo):
    import ml_dtypes
    f8 = ml_dtypes.float8_e4m3
    bf = ml_dtypes.bfloat16

    q = np.asarray(q, np.float32)
    k = np.asarray(k, np.float32)
    v = np.asarray(v, np.float32)
    Ws = {"Wq": np.asarray(Wq, np.float32), "Wk": np.asarray(Wk, np.float32),
          "Wv": np.asarray(Wv, np.float32), "Wo": np.asarray(Wo, np.float32)}
    bs = {"bq": np.asarray(bq, np.float32), "bk": np.asarray(bk, np.float32),
          "bv": np.asarray(bv, np.float32), "bo": np.asarray(bo, np.float32)}

    # shared across cores
    WT = {}
    for n_, w in Ws.items():
        WT[n_ + "T"] = np.ascontiguousarray(
            w.T.reshape(8, 128, 1024).transpose(1, 0, 2).reshape(128, 8192)
        ).astype(bf)
    bkT2 = np.ascontiguousarray(bs["bk"].reshape(8, 128).T)  # [128, 8]
    pidx = np.arange(128)
    onesk = np.zeros((128, 64), np.float32)
    for i in range(2):
        onesk[pidx, i * 32 + pidx // 4] = 1.0 / 64.0
    onesk = onesk.astype(f8)
    onesv = np.zeros((128, 32), np.float32)
    onesv[pidx, pidx // 4] = 1.0
    onesv = onesv.astype(bf)
    onesf = np.zeros((128, 32), np.float32)
    onesf[pidx, pidx % 32] = 1.0
    onesf = onesf.astype(bf)
    onesb = np.ascontiguousarray(onesf[:64].T).astype(bf)
    ident = np.eye(128, dtype=np.float32).astype(bf)

    in_maps = []
    for c in range(NCORES):
        b, half = c // 2, c % 2
        hs = slice(32 * half, 32 * half + 32)
        # kx rows g*128 + (4h+sa), cols hf*8192 + l*2048 + i*1024 + d,
        # with s = (2g+hf)*8 + sa*2 + i
        kc = k[b, hs]                                  # (32, 64, 4, 1024)
        kc = kc.reshape(32, 4, 2, 4, 2, 4, 1024)       # h, g, hf, sa, i, l, d
        kc = kc.transpose(1, 0, 3, 2, 5, 4, 6).reshape(512, 16384)
        # vx rows g*128 + (4h+sa), cols j*1024 + d, s = (g*4+j)*4 + sa
        vc = v[b, hs, :, L - 1, :]                     # (32, 64, 1024)
        vc = vc.reshape(32, 4, 4, 4, 1024)             # h, g, j, sa, d
        vc = vc.transpose(1, 0, 3, 2, 4).reshape(512, 4096)
        # qT: [dd, c*32 + r]
        qc = q[b, hs]                                  # (32, 1024)
        qTc = qc.T.reshape(8, 128, 32).transpose(1, 0, 2).reshape(128, 256)
        in_maps.append(dict(
            kx=np.ascontiguousarray(kc).astype(f8),
            vx=np.ascontiguousarray(vc).astype(bf),
            qT=np.ascontiguousarray(qTc).astype(bf),
            **WT, bq=bs["bq"], bkT2=bkT2, bv=bs["bv"], bo=bs["bo"],
            onesk=onesk, onesv=onesv, onesf=onesf, onesb=onesb,
            ident=ident))
    return in_maps


def assemble(results):
    out = np.empty((B, H, D), np.float32)
    for c in range(NCORES):
        b, half = c // 2, c % 2
        o = results[c]["out"]  # rows r = 2n + jp  ->  h' = 4n + 2*half + jp
        for r in range(32):
            out[b, 4 * (r // 2) + 2 * half + (r % 2)] = o[r]
    return out


def _install_ntff_shim():
    """Register the axon NTFF profile hook if the image's antenv lacks it."""
    import sys
    import types
    try:
        if "antenv.axon_hooks" in sys.modules:
            return
        import antenv
        mod = types.ModuleType("antenv.axon_hooks")
        mod._hook = None

        def set_axon_ntff_profile_hook(h):
            mod._hook = h

        def get_axon_ntff_profile_hook():
            return mod._hook

        mod.set_axon_ntff_profile_hook = set_axon_ntff_profile_hook
        mod.get_axon_ntff_profile_hook = get_axon_ntff_profile_hook
        sys.modules["antenv.axon_hooks"] = mod
        antenv.axon_hooks = mod
        from trn_agent_boot.trn_boot import _ntff_profile_via_ctypes
        hook = _ntff_profile_via_ctypes("/opt/axon/libaxon_pjrt.so")
        if hook is not None:
            set_axon_ntff_profile_hook(hook)
    except Exception:
        pass  # tracing degrades; execution unaffected


def kernel(q, k, v, Wq, bq, Wk, bk, Wv, bv, Wo, bo, _trace=False):
    global LAST_RESULTS
    from concourse.bass_utils import run_bass_kernel_spmd
    if _trace:
        _install_ntff_shim()
    nc = _get_nc()
    in_maps = make_in_maps(q, k, v, Wq, bq, Wk, bk, Wv, bv, Wo, bo)
    res = run_bass_kernel_spmd(nc, in_maps, list(range(NCORES)), trace=_trace)
    LAST_RESULTS = res
    return assemble(res.results)
